# revision 25
# baseline (speedup 1.0000x reference)
"""MeshGenLoss Trainium2 kernel.

Loss = chamfer(pred_pos, target_pos)
     + 0.001 * KL(mu, logvar)
     + 0.1   * mean_b std(nearest-neighbor-sqdist(pred_pos))
     + 0.05  * MSE(pred_sizing, target_sizing)

Sharding: 8 cores = (4 batches) x (2 row-halves of N=4096).

Each core computes three 2048x4096 distance matrices (pred->target,
target->pred, pred->pred with the diagonal masked) and row-min-reduces
them.  dist(i,j) = |x_i|^2 + (|y_j|^2 - 2 x_i.y_j); the parenthesised part
is computed as one K=21 bf16 matmul per 128-row block: both operands are
split into three bf16 limbs (24 mantissa bits) and the 6 significant limb
products plus the 3-limb |y|^2 row are summed by the PE, which reproduces
fp32 precision at bf16 matmul speed (fp32 matmuls run at 1/4 rate).  The
|x_i|^2 term is added after the row-min.

Row-min: ScalarE stages half of each PSUM block into SBUF, then one fused
min-min tensor_tensor_scan consumes (PSUM half, SBUF half); the scan's
last column is the running row minimum and chains across column passes
via its `initial` operand.  (tensor_tensor_reduce / tensor_mask_reduce
and all GPSIMD min/max tensor ops are rejected or crash this runtime -
the scan is the fast path that works; the cost model prices it at
~1.2us per 2048 elements and it measures ~2.3us on silicon.)

The Tile scheduler canonicalizes instruction order from dependencies
(emission order and priorities are ignored), so the steady state is a
zero-slack coupled pipeline: the ScalarE staging chain (1038+154ns per
pass) exactly matches the DVE scan chain (1192ns), and each row block
pays a ~0.3us slip where the first stage waits on its psA matmuls.  A
one-pass-ahead software-pipelined emission (pipelined=True) exists but
produces the identical schedule.

The diagonal of the self-distance matrix is excluded by streaming the
pred points in a per-core permuted order (own rows first) so the diagonal
always lands at compile-time-known columns, then accumulating BIG*I onto
that 128x128 block with one extra identity matmul.

Input DMAs are ordered first-needed-first: each wy1 replica's head
(stationary cols + first column pass) alternates between the two HWDGE
queues (SP / ScalarE); all later DMAs stay on SP so the ScalarE queue is
free for PSUM staging.

Per-core partial sums land in a [128, 68] stats tile; the host combines
the 8 outputs into the scalar loss.
"""

import numpy as np
import ml_dtypes

import concourse.bacc as bacc
import concourse.tile as tile
import concourse.mybir as mybir
from concourse.bass_utils import run_bass_kernel_spmd

P = 128           # partitions
N = 4096          # points per batch
R = 2048          # rows per core
NB = R // P       # 16 row blocks per matrix
HALF = 1024       # columns per PSUM half-tile (2 banks)
K = 21            # bf16 limb-product contraction depth
WY = R + N        # combined stationary|streaming tensor columns
BIG = 1e6
FMAX = 3.0e38

F32 = mybir.dt.float32
BF16 = mybir.dt.bfloat16
MIN = mybir.AluOpType.min
AX = mybir.AxisListType.X
BF = ml_dtypes.bfloat16

# stats tile columns
C_RM1 = 0     # 16 cols: row-min sums pred->target, per row block
C_RM2 = 16    # 16 cols: target->pred
C_RM3 = 32    # 16 cols: pred->pred (nearest neighbor)
C_RM3SQ = 48  # 16 cols: nn^2
C_SIZ = 64    # sizing squared-diff partial sum
C_MU2 = 65    # sum mu^2
C_LV = 66    # sum logvar
C_ELV = 67    # sum exp(logvar)
C_TOT = 68

# columns of the "smalls" input [P, 38]
SM_PSZ = 0    # 16: pred_sizing
SM_TSZ = 16   # 16: target_sizing
SM_MU = 32    # 2: mu
SM_LV = 34    # 2: logvar
SM_ELV = 36   # 2: exp(logvar)
SM_TOT = 38


def _build(cp_bufs=4, psa_bufs=2, psb_bufs=2, pipelined=False, reps=1):
    nc = bacc.Bacc(None, target_bir_lowering=False)

    wy1 = nc.dram_tensor("wy1", [K, WY], BF16, kind="ExternalInput")
    wy2 = nc.dram_tensor("wy2", [K, WY], BF16, kind="ExternalInput")
    wy3 = nc.dram_tensor("wy3", [K, WY], BF16, kind="ExternalInput")
    eyes = nc.dram_tensor("eyes", [P, 2 * P], BF16, kind="ExternalInput")
    x2s = nc.dram_tensor("x2s", [P, 48], F32, kind="ExternalInput")
    smalls = nc.dram_tensor("smalls", [P, SM_TOT], F32, kind="ExternalInput")
    out = nc.dram_tensor("out", [P, C_TOT], F32, kind="ExternalOutput")
    # passthrough used by the benchmark harness to chain sequential
    # executions inside one PJRT program (timing only; unused otherwise)
    tick = nc.dram_tensor("tick", [1, 1], F32, kind="ExternalInput")
    tock = nc.dram_tensor("tock", [1, 1], F32, kind="ExternalOutput")

    with tile.TileContext(nc) as tc:
        with (
            tc.tile_pool(name="const", bufs=1) as const_pool,
            tc.tile_pool(name="copies", bufs=cp_bufs) as cp_pool,
            tc.tile_pool(name="psum", bufs=2, space="PSUM") as psum_pool,
        ):
            wyr1 = const_pool.tile([P, WY], BF16)
            wyr2 = const_pool.tile([P, WY], BF16)
            wyr3 = const_pool.tile([P, WY], BF16)
            # Replicate the K=21 augmented operands at partition bases
            # 0/32/64/96 so four row-tiled matmuls can run concurrently.
            # First-needed data first: wy1 replica heads (stationary cols
            # + first column pass) alternate between the two HWDGE queues
            # (SP / ScalarE) so their transfers overlap; everything else
            # stays on SP so the ScalarE queue is free for PSUM staging.
            # The smalls tile comes early so the small-stats vector ops
            # can run during the pipeline ramp.
            head = R + 2048
            for j in range(4):
                eng = nc.sync if j % 2 == 0 else nc.scalar
                eng.dma_start(
                    out=wyr1[32 * j : 32 * j + K, 0:head], in_=wy1[:, 0:head]
                )
            for j in range(4):
                nc.sync.dma_start(
                    out=wyr1[32 * j : 32 * j + K, head:WY], in_=wy1[:, head:WY]
                )
            smt = const_pool.tile([P, SM_TOT], F32)
            nc.sync.dma_start(out=smt[:, :], in_=smalls[:, :])
            x2t = const_pool.tile([P, 48], F32)
            nc.sync.dma_start(out=x2t[:, :], in_=x2s[:, :])
            for j in range(4):
                nc.sync.dma_start(out=wyr2[32 * j : 32 * j + K, :], in_=wy2[:, :])
            for j in range(4):
                nc.sync.dma_start(out=wyr3[32 * j : 32 * j + K, :], in_=wy3[:, :])
            eyest = const_pool.tile([P, 2 * P], BF16)
            nc.sync.dma_start(out=eyest[:, :], in_=eyes[:, :])

            S = const_pool.tile([P, C_TOT], F32)
            RM = const_pool.tile([P, 48], F32)

            import contextlib

            loop_ctx = tc.For_i(0, reps, 1) if reps > 1 else contextlib.nullcontext()
            with loop_ctx:
                _emit_body(
                    nc, tc, cp_pool, psum_pool, const_pool,
                    wyr1, wyr2, wyr3, eyest, x2t, smt, S, RM,
                    psa_bufs, psb_bufs, pipelined,
                )

            nc.sync.dma_start(out=out[:, :], in_=S[:, :])
            nc.sync.dma_start(out=tock[:, :], in_=tick[:, :])

    nc.compile()
    return nc


def _emit_body(
    nc, tc, cp_pool, psum_pool, const_pool,
    wyr1, wyr2, wyr3, eyest, x2t, smt, S, RM,
    psa_bufs, psb_bufs, pipelined,
):
    # small statistics first: they only need the smalls tile, and fill
    # the DVE while the first wy DMAs land
    dsz = const_pool.tile([P, 16], F32)
    nc.vector.tensor_sub(
        dsz[:, :], smt[:, SM_PSZ : SM_PSZ + 16], smt[:, SM_TSZ : SM_TSZ + 16]
    )
    nc.vector.tensor_mul(dsz[:, :], dsz[:, :], dsz[:, :])
    nc.vector.reduce_sum(out=S[:, C_SIZ : C_SIZ + 1], in_=dsz[:, :], axis=AX)
    mu2 = const_pool.tile([P, 2], F32)
    nc.vector.tensor_mul(
        mu2[:, :], smt[:, SM_MU : SM_MU + 2], smt[:, SM_MU : SM_MU + 2]
    )
    nc.vector.reduce_sum(out=S[:, C_MU2 : C_MU2 + 1], in_=mu2[:, :], axis=AX)
    nc.vector.reduce_sum(
        out=S[:, C_LV : C_LV + 1], in_=smt[:, SM_LV : SM_LV + 2], axis=AX
    )
    nc.vector.reduce_sum(
        out=S[:, C_ELV : C_ELV + 1], in_=smt[:, SM_ELV : SM_ELV + 2], axis=AX
    )

    mats = (
        (wyr1, C_RM1, False),
        (wyr2, C_RM2, False),
        (wyr3, C_RM3, True),
    )
    passes = []
    for wyt, cbase, has_diag in mats:
        for rb in range(NB):
            for cp in range(2):
                passes.append((wyt, cbase, has_diag, rb, cp))

    def emit_A(pinfo, psA):
        wyt, cbase, has_diag, rb, cp = pinfo
        base = R + 2048 * cp
        # diagonal of row block rb lands at columns rb*128..rb*128+128
        # of column pass 0 (in psA for rb<8, in psB otherwise)
        diag_chunk = rb // 4 if (has_diag and cp == 0) else -1
        for i in (0, 1):
            nc.tensor.matmul(
                out=psA[:, (i % 2) * 512 : (i % 2) * 512 + 512],
                lhsT=wyt[32 * i : 32 * i + K, rb * P : (rb + 1) * P],
                rhs=wyt[32 * i : 32 * i + K, base + i * 512 : base + (i + 1) * 512],
                start=True,
                stop=(i != diag_chunk),
                tile_position=(32 * i, 0),
            )
            if i == 1 and diag_chunk >= 0 and rb < 8:
                nc.tensor.matmul(
                    out=psA[:, rb * P : rb * P + P],
                    lhsT=eyest[:, 0:P],
                    rhs=eyest[:, P : 2 * P],
                    start=False,
                    stop=True,
                )

    def emit_B(pinfo, psB):
        wyt, cbase, has_diag, rb, cp = pinfo
        base = R + 2048 * cp
        diag_chunk = rb // 4 if (has_diag and cp == 0) else -1
        for i in (2, 3):
            nc.tensor.matmul(
                out=psB[:, (i % 2) * 512 : (i % 2) * 512 + 512],
                lhsT=wyt[32 * i : 32 * i + K, rb * P : (rb + 1) * P],
                rhs=wyt[32 * i : 32 * i + K, base + i * 512 : base + (i + 1) * 512],
                start=True,
                stop=(i != diag_chunk),
                tile_position=(32 * i, 0),
            )
        if diag_chunk >= 0 and rb >= 8:
            nc.tensor.matmul(
                out=psB[:, rb * P - HALF : rb * P - HALF + P],
                lhsT=eyest[:, 0:P],
                rhs=eyest[:, P : 2 * P],
                start=False,
                stop=True,
            )

    so_prev = None
    if pipelined:
        # Software pipeline, one pass ahead (see module docstring).
        pend = None
        for k in range(len(passes) + 1):
            if k < len(passes):
                psA = psum_pool.tile(
                    [P, HALF], F32, tag="psA", name="psA", bufs=psa_bufs
                )
                psB = psum_pool.tile(
                    [P, HALF], F32, tag="psB", name="psB", bufs=psb_bufs
                )
                emit_A(passes[k], psA)
                sb = cp_pool.tile([P, HALF], F32, tag="sb", name="sb")
                nc.scalar.copy(out=sb[:, :], in_=psA[:, :])
            if k >= 1:
                pinfo_, psB_, sb_ = pend
                _, cbase_, _, rb_, cp_ = pinfo_
                emit_B(pinfo_, psB_)
                so = cp_pool.tile([P, HALF], F32, tag="so", name="so")
                init = FMAX if cp_ == 0 else so_prev[:, HALF - 1 :]
                nc.vector.tensor_tensor_scan(
                    out=so[:, :], data0=psB_[:, :], data1=sb_[:, :],
                    initial=init, op0=MIN, op1=MIN,
                )
                so_prev = so
                if cp_ == 1:
                    nc.gpsimd.tensor_copy(
                        out=RM[:, cbase_ + rb_ : cbase_ + rb_ + 1],
                        in_=so_prev[:, HALF - 1 :],
                    )
            if k < len(passes):
                pend = (passes[k], psB, sb)
    else:
        for pinfo in passes:
            _, cbase, _, rb, cp = pinfo
            psA = psum_pool.tile(
                [P, HALF], F32, tag="psA", name="psA", bufs=psa_bufs
            )
            psB = psum_pool.tile(
                [P, HALF], F32, tag="psB", name="psB", bufs=psb_bufs
            )
            emit_A(pinfo, psA)
            emit_B(pinfo, psB)
            # Alternate which half is staged: on cp==1 stage psB and scan
            # psA from PSUM (min-min is symmetric in its operands).  The
            # staged-side matmuls then always depend on a buffer that a
            # fast ScalarE stage released, never one a full scan held, so
            # the stage->matmul->stage chain has slack to absorb sem hops.
            staged, direct = (psA, psB) if cp == 0 else (psB, psA)
            sb = cp_pool.tile([P, HALF], F32, tag="sb", name="sb")
            nc.scalar.copy(out=sb[:, :], in_=staged[:, :])
            so = cp_pool.tile([P, HALF], F32, tag="so", name="so")
            init = FMAX if cp == 0 else so_prev[:, HALF - 1 :]
            nc.vector.tensor_tensor_scan(
                out=so[:, :], data0=direct[:, :], data1=sb[:, :],
                initial=init, op0=MIN, op1=MIN,
            )
            so_prev = so
            if cp == 1:
                eng = nc.vector if pinfo is passes[-1] else nc.gpsimd
                eng.tensor_copy(
                    out=RM[:, cbase + rb : cbase + rb + 1],
                    in_=so_prev[:, HALF - 1 :],
                )

    # add |x_row|^2 to all 48 block row-mins at once
    nc.vector.tensor_add(S[:, 0:48], RM[:, 0:48], x2t[:, 0:48])
    # nn^2 partials
    nc.vector.tensor_mul(
        S[:, C_RM3SQ : C_RM3SQ + 16],
        S[:, C_RM3 : C_RM3 + 16],
        S[:, C_RM3 : C_RM3 + 16],
    )


_NC_CACHE = None


def _get_nc():
    global _NC_CACHE
    if _NC_CACHE is None:
        _NC_CACHE = _build()
    return _NC_CACHE


def _split3(a):
    """fp32 array -> three bf16 limbs summing to a (to ~2^-27 rel)."""
    hi = a.astype(BF)
    r = a - hi.astype(np.float32)
    mid = r.astype(BF)
    r2 = r - mid.astype(np.float32)
    lo = r2.astype(BF)
    return hi, mid, lo


def _make_wy(x_rows, y_cols):
    """Stationary points x_rows [R,3], streaming points y_cols [N,3] ->
    [K, R+N] bf16: 6 limb-product pairs of x.(-2y) plus the 3-limb |y|^2."""
    xs = _split3(x_rows.T.astype(np.float32))          # [3, R] each
    z = -2.0 * y_cols.T.astype(np.float32)             # [3, N]
    zs = _split3(z)
    y2 = (y_cols.astype(np.float32) ** 2).sum(axis=1)  # [N]
    y2s = _split3(y2)

    w = np.zeros((K, R), dtype=BF)
    r = np.zeros((K, N), dtype=BF)
    pairs = [(0, 0), (0, 1), (1, 0), (0, 2), (2, 0), (1, 1)]
    for g, (si, mi) in enumerate(pairs):
        w[3 * g : 3 * g + 3] = xs[si]
        r[3 * g : 3 * g + 3] = zs[mi]
    w[18:21] = np.ones((3, R), dtype=BF)
    r[18] = y2s[0]
    r[19] = y2s[1]
    r[20] = y2s[2]
    return np.ascontiguousarray(np.concatenate([w, r], axis=1))


def _make_in_maps(pred_pos, pred_sizing, target_pos, target_sizing, mu, logvar):
    eyes = np.concatenate(
        [np.eye(P, dtype=BF), (np.eye(P) * BIG).astype(BF)], axis=1
    )
    explv = np.exp(logvar.astype(np.float32)).astype(np.float32)
    in_maps = []
    for c in range(8):
        b, h = c // 2, c % 2
        r0 = h * R
        pred = pred_pos[b]
        targ = target_pos[b]
        perm_pred = np.concatenate([pred[r0 : r0 + R], pred[: r0], pred[r0 + R :]])
        x2s = np.empty((P, 48), dtype=np.float32)
        x2s[:, 0:16] = (
            (pred[r0 : r0 + R].astype(np.float32) ** 2).sum(axis=1).reshape(NB, P).T
        )
        x2s[:, 16:32] = (
            (targ[r0 : r0 + R].astype(np.float32) ** 2).sum(axis=1).reshape(NB, P).T
        )
        x2s[:, 32:48] = x2s[:, 0:16]
        smalls = np.empty((P, SM_TOT), dtype=np.float32)
        smalls[:, SM_PSZ : SM_PSZ + 16] = pred_sizing[b, r0 : r0 + R].reshape(P, 16)
        smalls[:, SM_TSZ : SM_TSZ + 16] = target_sizing[b, r0 : r0 + R].reshape(P, 16)
        smalls[:, SM_MU : SM_MU + 2] = mu[b, h * 256 : (h + 1) * 256].reshape(P, 2)
        smalls[:, SM_LV : SM_LV + 2] = logvar[b, h * 256 : (h + 1) * 256].reshape(P, 2)
        smalls[:, SM_ELV : SM_ELV + 2] = explv[b, h * 256 : (h + 1) * 256].reshape(P, 2)
        in_maps.append(
            {
                "wy1": _make_wy(pred[r0 : r0 + R], targ),
                "wy2": _make_wy(targ[r0 : r0 + R], pred),
                "wy3": _make_wy(pred[r0 : r0 + R], perm_pred),
                "eyes": eyes,
                "x2s": x2s,
                "smalls": smalls,
                "tick": np.zeros((1, 1), dtype=np.float32),
            }
        )
    return in_maps


def _combine(outs):
    """outs: list of 8 [P, C_TOT] arrays -> scalar loss (float64 math)."""
    S = np.stack([o.astype(np.float64) for o in outs])  # [8, P, C_TOT]
    s1 = S[:, :, C_RM1 : C_RM1 + 16].sum(axis=(1, 2))     # per core
    s2 = S[:, :, C_RM2 : C_RM2 + 16].sum(axis=(1, 2))
    s3 = S[:, :, C_RM3 : C_RM3 + 16].sum(axis=(1, 2))
    s4 = S[:, :, C_RM3SQ : C_RM3SQ + 16].sum(axis=(1, 2))

    cd = (s1.sum() + s2.sum()) / (N * 4)

    density = 0.0
    for b in range(4):
        nn_sum = s3[2 * b] + s3[2 * b + 1]
        nn_sq = s4[2 * b] + s4[2 * b + 1]
        var = (nn_sq - nn_sum * nn_sum / N) / (N - 1)
        density += np.sqrt(max(var, 0.0)) / 4.0

    siz = S[:, :, C_SIZ].sum() / (4 * N)
    n_kl = 4 * 512
    kl = -0.5 * (
        1.0 + (S[:, :, C_LV].sum() - S[:, :, C_MU2].sum() - S[:, :, C_ELV].sum()) / n_kl
    )

    total = cd + 0.001 * kl + 0.1 * density + 0.05 * siz
    return np.float32(total)


def _run(inputs, trace=False, **kw):
    nc = _get_nc()
    in_maps = _make_in_maps(
        np.asarray(inputs["pred_pos"], dtype=np.float32),
        np.asarray(inputs["pred_sizing"], dtype=np.float32),
        np.asarray(inputs["target_pos"], dtype=np.float32),
        np.asarray(inputs["target_sizing"], dtype=np.float32),
        np.asarray(inputs["mu"], dtype=np.float32),
        np.asarray(inputs["logvar"], dtype=np.float32),
    )
    res = run_bass_kernel_spmd(nc, in_maps, core_ids=list(range(8)), trace=trace, **kw)
    total = _combine([r["out"] for r in res.results])
    return total, res


def kernel(**inputs):
    return _run(inputs)[0]
